# revision 1
# baseline (speedup 1.0000x reference)
"""Trainium2 Bass kernel for the MeSH GCN+CNN model, distributed over 8
NeuronCores.

Strategy:
  - GCN: nodes are permuted and bin-packed into 128-node blocks balanced by
    in-degree, 49 blocks per core (vertex partition by dst).  Segment-sum
    aggregation is computed as a sequence of small matmuls: for each chunk of
    128 edges, gathered source features (dma_gather, bf16) are multiplied by
    an on-the-fly one-hot (edge -> local dst) matrix, accumulating agg^T in
    PSUM.  Linear layers run in fp32r at full PE rate.  h is exchanged with
    an 8-core AllGather so layer 2 can gather any source row locally.
  - CNN: embedding lookup is done host-side (indices are host-known); convs
    are evaluated as matmuls with weights stationary, outputs (o x t), and
    relu/max-pool collapses to a free-dim max reduction of PSUM.
  - log_softmax runs on-device per 128-row block.
"""

import math
import heapq

import numpy as np
import ml_dtypes

import concourse.bass as bass
import concourse.mybir as mybir
import concourse.tile as tile
from concourse.bass_utils import run_bass_kernel_spmd
from concourse.library_config import mlp
from concourse.tile import TileContext, ScopedClock

dt = mybir.dt
BF16 = ml_dtypes.bfloat16
_REAL_RUNNER = run_bass_kernel_spmd

# ---------------------------------------------------------------------------
# Workarounds for this container's walrus build: at most ONE sync-wait
# command per instruction.  (1) Tile's tail drain carries one wait per
# logical processor -> redistribute over single-wait NOPs.  (2) After
# scheduling, split any instruction with >1 waits.
# ---------------------------------------------------------------------------


def _drain_and_barrier(self, tick_clock, wait_clock):
    nc = self.nc
    probe = nc.sync.nop(nofuse=True, hint="tail_wait_probe")
    wait_clock.add_sem_waits(probe.ins, ScopedClock({None: tick_clock.global_clock}))
    si = probe.ins.sync_info
    waits = list(si.on_wait) if si and si.on_wait else []
    if si is not None:
        si.on_wait = []
    for i, w in enumerate(waits):
        nop = nc.sync.nop(nofuse=True, hint=f"tail_waits_{i}")
        nop.ins.sync_info = mybir.SyncInfo(on_wait=[w], on_update=[])
    nc.sync.drain()
    nc.all_engine_barrier()
    popped = nc._tile_sem_poison_stack.pop()
    assert popped is self._sem_poison
    nc.clear_and_free_semaphores(list(self.sems.allocated().values()))
    nc.all_engine_barrier()


TileContext._drain_and_barrier = _drain_and_barrier


def _split_multi_waits(nc):
    for fn in nc.m.functions:
        for bb in fn.blocks:
            insts = list(bb.instructions)
            out = []
            changed = False
            for inst in insts:
                si = inst.sync_info
                waits = list(si.on_wait) if si is not None and si.on_wait else []
                if len(waits) > 1:
                    changed = True
                    for w in waits[:-1]:
                        nop = mybir.InstNoOp(
                            name=f"waitsplit_{nc.next_id()}", engine=inst.engine
                        )
                        nop.sync_info = mybir.SyncInfo(on_wait=[w], on_update=[])
                        nc.register_instruction(nop, overwrite=True)
                        out.append(nop)
                    si.on_wait = [waits[-1]]
                out.append(inst)
            if changed:
                bb.instructions = out


# ---------------------------------------------------------------------------
# Configuration
# ---------------------------------------------------------------------------


class CFG:
    def __init__(self, **kw):
        self.n_cores = 8
        self.n_nodes = 50000
        self.nblk_pc = 49          # 128-node blocks per core
        self.sbb = 6               # blocks per superblock
        self.split = 32768         # int16 index limit for dma_gather
        self.f1 = 128
        self.f2 = 256
        self.ncls = 768
        self.emb_d = 200
        self.t_len = 2048
        self.spc = 4               # sequences per core (batch 32 / 8)
        self.ks = (3, 4, 5)
        self.__dict__.update(kw)
        rem = self.nblk_pc % self.sbb
        self.sb_sizes = [self.sbb] * (self.nblk_pc // self.sbb) + (
            [rem] if rem else [])
        self.nsb = len(self.sb_sizes)
        self.rows_pc = self.nblk_pc * 128
        self.perm_n = self.n_cores * self.rows_pc
        self.tp = self.t_len + 4          # zero-padded time axis
        self.tsup = self.t_len // 512     # 512-wide t supertiles
        assert self.t_len % 512 == 0
        # CNN o-blocks: 2 blocks of 128 outputs per conv k
        self.nob = 2 * len(self.ks)
        assert self.ncls == self.nob * 128


# ---------------------------------------------------------------------------
# Host-side planning
# ---------------------------------------------------------------------------


def _host_plan(cfg, edge_src, edge_dst):
    """Permute nodes into balanced 128-node blocks; build per-core chunked
    edge metadata (gather indices + local dst)."""
    N = cfg.n_nodes
    nblk_total = cfg.n_cores * cfg.nblk_pc
    deg = np.bincount(edge_dst, minlength=N).astype(np.int64)

    order = np.argsort(-deg, kind="stable")
    heap = [(0, b) for b in range(nblk_total)]
    heapq.heapify(heap)
    cap = np.zeros(nblk_total, np.int32)
    load = np.zeros(nblk_total, np.int64)
    blk_of = np.empty(N, np.int32)
    row_of = np.empty(N, np.int32)
    for n in order:
        while True:
            _, b = heapq.heappop(heap)
            if cap[b] < 128:
                break
        blk_of[n] = b
        row_of[n] = cap[b]
        cap[b] += 1
        load[b] += deg[n]
        if cap[b] < 128:
            heapq.heappush(heap, (load[b], b))

    core_of_blk = np.arange(nblk_total) // cfg.nblk_pc
    blk_in_core = np.arange(nblk_total) % cfg.nblk_pc
    pid = (
        core_of_blk[blk_of] * cfg.rows_pc + blk_in_core[blk_of] * 128 + row_of
    ).astype(np.int64)

    s_pid = pid[edge_src]
    d_blk = blk_of[edge_dst]          # global block id of each edge's dst
    d_row = row_of[edge_dst]
    is_b = s_pid >= cfg.split

    # counts per (block, section)
    nA = np.bincount(d_blk[~is_b], minlength=nblk_total)
    nB = np.bincount(d_blk[is_b], minlength=nblk_total)
    K_A = max(1, int(np.ceil(nA.max() / 128)))
    K_B = max(1, int(np.ceil(nB.max() / 128)))
    kch = K_A + K_B
    totch = cfg.nblk_pc * kch
    sbch = cfg.sbb * kch

    # global slot of edge e:  per core layout of superblocks:
    #   sb base = sb*sbb*kch; A slots [base, base+sbb*K_A) block-major,
    #   B slots [base+sbb*K_A, base+sbb*kch)
    # position within a (block, section): stable cumcount
    sec = is_b.astype(np.int64)
    key = d_blk.astype(np.int64) * 2 + sec
    sort_idx = np.argsort(key, kind="stable")
    ks = key[sort_idx]
    # cumcount within sorted groups
    first = np.r_[True, ks[1:] != ks[:-1]]
    gstart = np.zeros(len(ks), np.int64)
    gstart[first] = np.arange(len(ks))[first]
    gstart = np.maximum.accumulate(gstart)
    pos_sorted = np.arange(len(ks)) - gstart
    pos = np.empty(len(ks), np.int64)
    pos[sort_idx] = pos_sorted

    blk_local = (d_blk % cfg.nblk_pc).astype(np.int64)
    sb = blk_local // cfg.sbb
    j = blk_local % cfg.sbb
    base = sb * sbch                          # full SBs precede any partial one
    sb_size = np.asarray(cfg.sb_sizes, np.int64)[sb]
    slotA = base + j * K_A + pos // 128
    slotB = base + sb_size * K_A + j * K_B + pos // 128
    slot = np.where(is_b, slotB, slotA)       # slot within the core
    lane = pos % 128

    core = (d_blk // cfg.nblk_pc).astype(np.int64)

    idx_val = np.where(is_b, s_pid - cfg.split, s_pid).astype(np.int16)

    idx_all = []
    dstl_all = []
    for c in range(cfg.n_cores):
        m = core == c
        idx_sm = np.zeros((totch, 128), np.int16)
        dstl_sm = np.full((totch, 128), -1.0, np.float32)
        idx_sm[slot[m], lane[m]] = idx_val[m]
        dstl_sm[slot[m], lane[m]] = d_row[m].astype(np.float32)
        # wrap for dma_gather: j -> (partition j%16, col j//16), slot-major
        wrapped = idx_sm.reshape(totch, 8, 16).transpose(2, 0, 1).reshape(16, totch * 8)
        idx_all.append(np.tile(wrapped, (8, 1)))          # replicate to 128 parts
        dstl_all.append(dstl_sm.T.astype(BF16).copy())    # (128, totch)

    return dict(
        pid=pid, K_A=K_A, K_B=K_B, kch=kch, totch=totch, sbch=sbch,
        idx=idx_all, dstl=dstl_all,
    )


def _host_cnn_prep(cfg, input_seq, emb, conv_ws, conv_bs):
    emb_bf = emb.astype(BF16)
    xTs = []
    for c in range(cfg.n_cores):
        seqs = input_seq[c * cfg.spc:(c + 1) * cfg.spc]
        x = emb_bf[seqs]                                   # (spc, T, emb_d)
        xT = np.zeros((cfg.emb_d, cfg.spc * cfg.tp), BF16)
        for s in range(cfg.spc):
            xT[:, s * cfg.tp: s * cfg.tp + cfg.t_len] = x[s].T
        xTs.append(xT)
    ndt = max(cfg.ks)
    wcat = np.zeros((cfg.emb_d, ndt * cfg.ncls), np.float32)
    bcat = np.zeros((128, cfg.nob), np.float32)
    for ki, k in enumerate(cfg.ks):
        w = conv_ws[ki]      # (256,1,k,emb_d)
        b = conv_bs[ki]      # (256,)
        o0 = ki * 256
        for dti in range(k):
            wcat[:, dti * cfg.ncls + o0: dti * cfg.ncls + o0 + 256] = w[:, 0, dti, :].T
        bcat[:, 2 * ki] = b[:128]
        bcat[:, 2 * ki + 1] = b[128:]
    wlo = wcat[:128].astype(BF16)
    whi_p = cfg.emb_d - 128
    whi = wcat[128:].astype(BF16)
    return xTs, wlo, whi, whi_p, bcat, ndt


# ---------------------------------------------------------------------------
# Device program (uniform across cores)
# ---------------------------------------------------------------------------


def _build_program(cfg, K_A, K_B):
    f32, f32r, bf16, i16 = dt.float32, dt.float32r, dt.bfloat16, dt.int16
    kch = K_A + K_B
    totch = cfg.nblk_pc * kch
    sbch = cfg.sbb * kch
    ndt = max(cfg.ks)
    whi_p = cfg.emb_d - 128

    nc = bass.Bass("TRN2", target_bir_lowering=False, debug=False,
                   num_devices=cfg.n_cores)

    # -------- I/O --------
    feat = nc.dram_tensor("feat", [cfg.perm_n, cfg.f1], bf16, kind="ExternalInput").ap()
    idx = nc.dram_tensor("idx", [128, totch * 8], i16, kind="ExternalInput").ap()
    dstl = nc.dram_tensor("dstl", [128, totch], bf16, kind="ExternalInput").ap()
    iota = nc.dram_tensor("iota", [128, 128], bf16, kind="ExternalInput").ap()
    w1 = nc.dram_tensor("w1", [cfg.f1, cfg.f2], bf16, kind="ExternalInput").ap()
    b1r = nc.dram_tensor("b1r", [1, cfg.f2], bf16, kind="ExternalInput").ap()
    w2r = nc.dram_tensor("w2r", [128, 2 * cfg.ncls], bf16, kind="ExternalInput").ap()
    b2r = nc.dram_tensor("b2r", [1, cfg.ncls], bf16, kind="ExternalInput").ap()
    ones = nc.dram_tensor("ones", [1, 128], bf16, kind="ExternalInput").ap()
    xT = nc.dram_tensor("xT", [cfg.emb_d, cfg.spc * cfg.tp], bf16,
                        kind="ExternalInput").ap()
    wlo = nc.dram_tensor("wlo", [128, ndt * cfg.ncls], bf16, kind="ExternalInput").ap()
    whi = nc.dram_tensor("whi", [whi_p, ndt * cfg.ncls], bf16,
                         kind="ExternalInput").ap()
    bcat = nc.dram_tensor("bcat", [128, cfg.nob], f32, kind="ExternalInput").ap()

    label_ls = nc.dram_tensor("label_ls", [cfg.rows_pc, cfg.ncls], f32,
                              kind="ExternalOutput").ap()
    cnn_ls = nc.dram_tensor("cnn_ls", [cfg.spc, cfg.ncls], f32,
                            kind="ExternalOutput").ap()

    cc_in = nc.dram_tensor("cc_in", [cfg.rows_pc, cfg.f2], bf16).ap()
    cc_out = nc.dram_tensor("cc_out", [cfg.perm_n, cfg.f2], bf16,
                            addr_space="Shared").ap()
    cnn_feat = nc.dram_tensor("cnn_feat", [cfg.spc * cfg.nob, 128], f32).ap()

    nc.gpsimd.load_library(mlp)

    sb_base = [0]
    for n in cfg.sb_sizes:
        sb_base.append(sb_base[-1] + n * kch)

    def slot_block(sb, s):
        """slot within superblock sb -> (block index in sb, start?, stop?)"""
        nsbb = cfg.sb_sizes[sb]
        if s < nsbb * K_A:
            b, a = divmod(s, K_A)
            return b, a == 0, False
        s2 = s - nsbb * K_A
        b, bb = divmod(s2, K_B)
        return b, False, bb == K_B - 1

    with TileContext(nc) as tc:
        with tc.tile_pool(name="persist", bufs=1) as pp:
            idx_t = pp.tile([128, totch * 8], i16)
            nc.sync.dma_start(out=idx_t[:], in_=idx[:])
            dstl_t = pp.tile([128, totch], bf16)
            nc.sync.dma_start(out=dstl_t[:], in_=dstl[:])
            iota_t = pp.tile([128, 128], bf16)
            nc.sync.dma_start(out=iota_t[:], in_=iota[:])
            w1_t = pp.tile([cfg.f1, cfg.f2], bf16)
            nc.sync.dma_start(out=w1_t[:], in_=w1[:])
            b1_t = pp.tile([1, cfg.f2], bf16)
            nc.sync.dma_start(out=b1_t[:], in_=b1r[:])
            w2_t = pp.tile([128, 2 * cfg.ncls], bf16)
            nc.sync.dma_start(out=w2_t[:], in_=w2r[:])
            b2_t = pp.tile([1, cfg.ncls], bf16)
            nc.sync.dma_start(out=b2_t[:], in_=b2r[:])
            ones_t = pp.tile([1, 128], bf16)
            nc.sync.dma_start(out=ones_t[:], in_=ones[:])
            wlo_t = pp.tile([128, ndt * cfg.ncls], bf16)
            nc.sync.dma_start(out=wlo_t[:], in_=wlo[:])
            whi_t = pp.tile([whi_p, ndt * cfg.ncls], bf16)
            nc.sync.dma_start(out=whi_t[:], in_=whi[:])
            bcat_t = pp.tile([128, cfg.nob], f32)
            nc.sync.dma_start(out=bcat_t[:], in_=bcat[:])

            def iota_rep(n):
                return bass.AP(iota_t[:].tensor, iota_t[:].offset,
                               [iota_t[:].ap[0], [0, n], [1, 128]])

            def onehot_ops(pool, sb):
                """one-hot tiles for a whole superblock (shape 128 x n x 128)"""
                n = cfg.sb_sizes[sb] * kch
                oh = pool.tile([128, sbch, 128], bf16, tag="oh")
                s0 = sb_base[sb]
                d = dstl_t[:, s0:s0 + n].to_broadcast([128, n, 128])
                nc.vector.tensor_tensor(out=oh[:, 0:n, :], in0=d,
                                        in1=iota_rep(n),
                                        op=mybir.AluOpType.is_equal)
                return oh

            def gathers(pool, sb, table, felem, tag):
                g = pool.tile([128, sbch, felem], bf16, tag=tag)
                s0 = sb_base[sb]
                na = cfg.sb_sizes[sb] * K_A
                nc.gpsimd.dma_gather(
                    out_ap=g[:, 0:na, :], in_ap=table[0:cfg.split, :],
                    idxs_ap=idx_t[:, s0 * 8:(s0 + na) * 8],
                    num_idxs=na * 128, num_idxs_reg=na * 128,
                    elem_size=felem, single_packet=False)
                nb = cfg.sb_sizes[sb] * K_B
                nc.gpsimd.dma_gather(
                    out_ap=g[:, na:na + nb, :], in_ap=table[cfg.split:cfg.perm_n, :],
                    idxs_ap=idx_t[:, (s0 + na) * 8:(s0 + na + nb) * 8],
                    num_idxs=nb * 128, num_idxs_reg=nb * 128,
                    elem_size=felem, single_packet=False)
                return g

            # ---------------- GCN layer 1 ----------------
            with tc.tile_pool(name="l1", bufs=2) as lp, \
                 tc.tile_pool(name="l1ps", bufs=1, space="PSUM") as ps1, \
                 tc.tile_pool(name="l1psh", bufs=2, space="PSUM") as psh:
                for sb in range(cfg.nsb):
                    g = gathers(lp, sb, feat, cfg.f1, "g1")
                    oh = onehot_ops(lp, sb)
                    aggps = [ps1.tile([128, 128], f32, space="PSUM", tag=f"agg1_{b}",
                                      name=f"agg1_{sb}_{b}")
                             for b in range(cfg.sb_sizes[sb])]
                    for s in range(cfg.sb_sizes[sb] * kch):
                        b, st, sp = slot_block(sb, s)
                        nc.tensor.matmul(out=aggps[b][:], lhsT=g[:, s, :],
                                         rhs=oh[:, s, :], start=st, stop=sp)
                    for b in range(cfg.sb_sizes[sb]):
                        blk = sb * cfg.sbb + b
                        aggt = lp.tile([128, 128], bf16, tag="aggt")
                        nc.vector.tensor_copy(out=aggt[:], in_=aggps[b][:])
                        hps = psh.tile([128, cfg.f2], f32, space="PSUM", tag="hps")
                        nc.tensor.matmul(out=hps[:], lhsT=aggt[:],
                                         rhs=w1_t[:],
                                         start=True, stop=False)
                        nc.tensor.matmul(out=hps[:], lhsT=ones_t[:],
                                         rhs=b1_t[:],
                                         start=False, stop=True)
                        hsb = lp.tile([128, cfg.f2], bf16, tag="hsb")
                        nc.scalar.activation(out=hsb[:], in_=hps[:],
                                             func=mybir.ActivationFunctionType.Relu)
                        nc.sync.dma_start(out=cc_in[blk * 128:(blk + 1) * 128, :],
                                          in_=hsb[:])

            # ---------------- AllGather h ----------------
            nc.gpsimd.collective_compute(
                "AllGather", mybir.AluOpType.bypass,
                ins=[cc_in[:]], outs=[cc_out[:]],
                replica_groups=[list(range(cfg.n_cores))])

            # ---------------- CNN ----------------
            with tc.tile_pool(name="cnn", bufs=2) as cp, \
                 tc.tile_pool(name="cnnps", bufs=1, space="PSUM") as cps:
                for s in range(cfg.spc):
                    xlo = cp.tile([128, cfg.tp], bf16, tag="xlo")
                    nc.sync.dma_start(out=xlo[:],
                                      in_=xT[0:128, s * cfg.tp:(s + 1) * cfg.tp])
                    xhi = cp.tile([whi_p, cfg.tp], bf16, tag="xhi")
                    nc.sync.dma_start(out=xhi[:],
                                      in_=xT[128:cfg.emb_d, s * cfg.tp:(s + 1) * cfg.tp])
                    for ob in range(cfg.nob):
                        k = cfg.ks[ob // 2]
                        pcs = [cps.tile([128, 512], f32, space="PSUM", tag=f"cnnp{t}",
                                        name=f"cnnp_{s}_{ob}_{t}")
                               for t in range(cfg.tsup)]
                        for dti in range(k):
                            for wi, (wt, xt, np_) in enumerate(
                                    ((wlo_t, xlo, 128), (whi_t, xhi, whi_p))):
                                lhs = wt[:, dti * cfg.ncls + ob * 128:
                                         dti * cfg.ncls + (ob + 1) * 128]
                                for t in range(cfg.tsup):
                                    nc.tensor.matmul(
                                        out=pcs[t][:],
                                        lhsT=lhs,
                                        rhs=xt[:, t * 512 + dti: t * 512 + dti + 512],
                                        start=(dti == 0 and wi == 0),
                                        stop=(dti == k - 1 and wi == 1))
                        cm = cp.tile([128, cfg.tsup], f32, tag="cm")
                        for t in range(cfg.tsup):
                            vl = min(512, cfg.t_len - k + 1 - t * 512)
                            nc.vector.tensor_reduce(
                                out=cm[:, t:t + 1], in_=pcs[t][:, 0:vl],
                                axis=mybir.AxisListType.X,
                                op=mybir.AluOpType.max)
                        xf = cp.tile([128, 1], f32, tag="xf")
                        nc.vector.tensor_reduce(
                            out=xf[:], in_=cm[:], axis=mybir.AxisListType.X,
                            op=mybir.AluOpType.max)
                        xfr = cp.tile([128, 1], f32, tag="xfr")
                        nc.scalar.activation(out=xfr[:], in_=xf[:],
                                             func=mybir.ActivationFunctionType.Relu,
                                             bias=bcat_t[:, ob:ob + 1])
                        nc.sync.dma_start(
                            out=cnn_feat[s * cfg.nob + ob, :],
                            in_=xfr[:, 0:1])

            # ---------------- GCN layer 2 + log_softmax ----------------
            def log_softmax(pool, lab, nrows, out_ap):
                """lab: SBUF tile (nrows, ncls) f32 -> out_ap DRAM rows"""
                nmax = pool.tile([128, 1], f32, tag="nmax")
                nc.vector.tensor_reduce(out=nmax[:nrows], in_=lab[:nrows],
                                        axis=mybir.AxisListType.X,
                                        op=mybir.AluOpType.max, negate=True)
                esc = pool.tile([128, cfg.ncls], f32, tag="esc")
                sume = pool.tile([128, 1], f32, tag="sume")
                nc.scalar.activation(out=esc[:nrows], in_=lab[:nrows],
                                     func=mybir.ActivationFunctionType.Exp,
                                     bias=nmax[:nrows], accum_out=sume[:nrows])
                lz = pool.tile([128, 1], f32, tag="lz")
                nc.scalar.activation(out=lz[:nrows], in_=sume[:nrows],
                                     func=mybir.ActivationFunctionType.Ln)
                sh = pool.tile([128, 1], f32, tag="sh")
                nc.vector.tensor_sub(out=sh[:nrows], in0=nmax[:nrows],
                                     in1=lz[:nrows])
                ols = pool.tile([128, cfg.ncls], f32, tag="ols")
                nc.vector.tensor_scalar(out=ols[:nrows], in0=lab[:nrows],
                                        scalar1=sh[:nrows], scalar2=None,
                                        op0=mybir.AluOpType.add)
                nc.sync.dma_start(out=out_ap, in_=ols[:nrows])

            with tc.tile_pool(name="l2", bufs=2) as lp2, \
                 tc.tile_pool(name="l2ps", bufs=1, space="PSUM") as ps2, \
                 tc.tile_pool(name="l2psl", bufs=1, space="PSUM") as psl:
                for sb in range(cfg.nsb):
                    g = gathers(lp2, sb, cc_out, cfg.f2, "g2")
                    oh = onehot_ops(lp2, sb)
                    # two feature halves as sequential passes over shared banks
                    aggps = [ps2.tile([128, 128], f32, space="PSUM", tag=f"a2_{b}",
                                      name=f"a2_{sb}_{b}")
                             for b in range(cfg.sb_sizes[sb])]
                    a2h = [[], []]
                    for half in range(2):
                        c0 = half * 128
                        for s in range(cfg.sb_sizes[sb] * kch):
                            b, st, sp = slot_block(sb, s)
                            nc.tensor.matmul(out=aggps[b][:],
                                             lhsT=g[:, s, c0:c0 + 128],
                                             rhs=oh[:, s, :], start=st, stop=sp)
                        for b in range(cfg.sb_sizes[sb]):
                            t = lp2.tile([128, 128], bf16, tag=f"a2h{half}_{b}",
                                         name=f"a2h_{sb}_{b}_{half}")
                            nc.vector.tensor_copy(out=t[:], in_=aggps[b][:])
                            a2h[half].append(t)
                    for b in range(cfg.sb_sizes[sb]):
                        blk = sb * cfg.sbb + b
                        a2a = a2h[0][b]
                        a2b = a2h[1][b]
                        lps = [psl.tile([128, 384], f32, space="PSUM", tag=f"lp{h}",
                                        name=f"lp_{sb}_{b}_{h}")
                               for h in range(2)]
                        for h in range(2):
                            col = h * 384
                            nc.tensor.matmul(
                                out=lps[h][:], lhsT=a2a[:],
                                rhs=w2_t[:, col:col + 384],
                                start=True, stop=False)
                            nc.tensor.matmul(
                                out=lps[h][:], lhsT=a2b[:],
                                rhs=w2_t[:, cfg.ncls + col:cfg.ncls + col + 384],
                                start=False, stop=False)
                            nc.tensor.matmul(
                                out=lps[h][:], lhsT=ones_t[:],
                                rhs=b2_t[:, col:col + 384],
                                start=False, stop=True)
                        lab = lp2.tile([128, cfg.ncls], f32, tag="lab")
                        nc.vector.tensor_copy(out=lab[:, 0:384], in_=lps[0][:])
                        nc.vector.tensor_copy(out=lab[:, 384:768], in_=lps[1][:])
                        log_softmax(lp2, lab, 128,
                                    label_ls[blk * 128:(blk + 1) * 128, :])

                # CNN rows log_softmax
                cf = lp2.tile([cfg.spc, cfg.ncls], f32, tag="cf")
                cnn_feat_rows = bass.AP(cnn_feat.tensor, 0,
                                        [[cfg.ncls, cfg.spc], [1, cfg.ncls]])
                nc.sync.dma_start(out=cf[:], in_=cnn_feat_rows)
                log_softmax(lp2, cf, cfg.spc, cnn_ls[:, :])

    mybir.codegen_inst_isa_subclasses(nc)
    _split_multi_waits(nc)
    return nc


# ---------------------------------------------------------------------------
# Execution (timed variant of bass2jax.run_bass_via_pjrt)
# ---------------------------------------------------------------------------


def _execute(nc, in_maps, n_cores, bench=0):
    """Run the SPMD module on n_cores axon devices.  With bench>0, re-runs
    the cached executable and returns (results, best_wall_seconds)."""
    import time
    import jax
    from jax.sharding import Mesh, PartitionSpec, NamedSharding
    from jax.experimental.shard_map import shard_map
    from concourse import bass2jax

    bass2jax.install_neuronx_cc_hook()
    partition_name = nc.partition_id_tensor.name if nc.partition_id_tensor else None

    in_names, out_names, out_avals, zero_outs = [], [], [], []
    for alloc in nc.m.functions[0].allocations:
        if not isinstance(alloc, mybir.MemoryLocationSet):
            continue
        name = alloc.memorylocations[0].name
        if alloc.kind == "ExternalInput":
            if name != partition_name:
                in_names.append(name)
        elif alloc.kind == "ExternalOutput":
            shape = tuple(alloc.tensor_shape)
            np_dt = dt.np(alloc.dtype)
            out_names.append(name)
            out_avals.append(jax.core.ShapedArray(shape, np_dt))
            zero_outs.append(np.zeros(shape, np_dt))
    n_params = len(in_names)
    in_names_all = in_names + out_names
    if partition_name is not None:
        in_names_all.append(partition_name)

    def _mk_body(chain):
        def _body(*args):
            ins = list(args[:n_params])
            cur = list(args[n_params:])
            for _ in range(chain):
                operands = ins + cur
                if partition_name is not None:
                    operands.append(bass2jax.partition_id_tensor())
                cur = list(bass2jax._bass_exec_p.bind(
                    *operands,
                    out_avals=tuple(out_avals),
                    in_names=tuple(in_names_all),
                    out_names=tuple(out_names),
                    lowering_input_output_aliases=(),
                    sim_require_finite=True,
                    sim_require_nnan=True,
                    nc=nc,
                ))
            return tuple(cur)
        return _body

    devices = jax.devices()[:n_cores]
    mesh = Mesh(np.asarray(devices), ("core",))
    spec = PartitionSpec("core")
    in_specs = (spec,) * (n_params + len(out_names))
    out_specs = (spec,) * len(out_names)

    def _mk_sharded(chain):
        return jax.jit(
            shard_map(_mk_body(chain), mesh=mesh, in_specs=in_specs,
                      out_specs=out_specs, check_rep=False),
            keep_unused=True,
        )

    sharded = _mk_sharded(1)
    sh = NamedSharding(mesh, spec)
    concat_in = [
        jax.device_put(
            np.concatenate([np.asarray(in_maps[c][n]) for c in range(n_cores)], 0),
            sh)
        for n in in_names
    ]
    concat_zeros = [
        jax.device_put(np.zeros((n_cores * z.shape[0], *z.shape[1:]), z.dtype), sh)
        for z in zero_outs
    ]
    out_arrs = sharded(*concat_in, *concat_zeros)
    jax.block_until_ready(out_arrs)
    best = None
    if bench:
        # chained executions inside one dispatch: outputs feed the next
        # call's output-buffer operands, forcing serial execution; the
        # wall-time slope vs chain length isolates per-execution HW time.
        CH = 8
        chained = _mk_sharded(CH)
        o2 = chained(*concat_in, *concat_zeros)
        jax.block_until_ready(o2)

        def timeit(fn, n):
            ts = []
            for _ in range(n):
                t0 = time.perf_counter()
                jax.block_until_ready(fn(*concat_in, *concat_zeros))
                ts.append(time.perf_counter() - t0)
            return min(ts)

        t1 = timeit(sharded, bench)
        tc_ = timeit(chained, bench)
        best = (tc_ - t1) / (CH - 1)
        print(f"[bench] wall chain1={t1*1e3:.2f} ms  chain{CH}={tc_*1e3:.2f} ms"
              f" -> per-exec {best*1e3:.3f} ms")
    results = [
        {n: np.asarray(out_arrs[i]).reshape(n_cores, *out_avals[i].shape)[c]
         for i, n in enumerate(out_names)}
        for c in range(n_cores)
    ]
    return results, best


# ---------------------------------------------------------------------------
# kernel()
# ---------------------------------------------------------------------------


def kernel(input_seq, edge_src, edge_dst, features, emb,
           conv_w3, conv_b3, conv_w4, conv_b4, conv_w5, conv_b5,
           gcn1_w, gcn1_b, gcn2_w, gcn2_b, cfg=None, bench=0, _out=[None]):
    cfg = cfg or CFG()
    input_seq = np.asarray(input_seq)
    edge_src = np.asarray(edge_src).astype(np.int64)
    edge_dst = np.asarray(edge_dst).astype(np.int64)
    features = np.asarray(features, dtype=np.float32)
    emb = np.asarray(emb, dtype=np.float32)

    plan = _host_plan(cfg, edge_src, edge_dst)
    pid = plan["pid"]

    feat_perm = np.zeros((cfg.perm_n, cfg.f1), BF16)
    feat_perm[pid] = features.astype(BF16)

    xTs, wlo, whi, whi_p, bcat, ndt = _host_cnn_prep(
        cfg, input_seq, emb,
        [conv_w3, conv_w4, conv_w5], [conv_b3, conv_b4, conv_b5])

    iota = np.tile(np.arange(128, dtype=np.float32), (128, 1)).astype(BF16)
    w2r = np.zeros((128, 2 * cfg.ncls), np.float32)
    w2r[:, 0:cfg.ncls] = gcn2_w[0:128]
    w2r[:, cfg.ncls:] = gcn2_w[128:256]

    nc = _build_program(cfg, plan["K_A"], plan["K_B"])

    shared = dict(
        feat=feat_perm, iota=iota,
        w1=np.asarray(gcn1_w, np.float32).astype(BF16),
        b1r=np.asarray(gcn1_b, np.float32).reshape(1, -1).astype(BF16),
        w2r=w2r.astype(BF16),
        b2r=np.asarray(gcn2_b, np.float32).reshape(1, -1).astype(BF16),
        ones=np.ones((1, 128), BF16),
        wlo=wlo, whi=whi, bcat=bcat,
    )
    in_maps = []
    for c in range(cfg.n_cores):
        m = dict(shared)
        m["idx"] = plan["idx"][c]
        m["dstl"] = plan["dstl"][c]
        m["xT"] = xTs[c]
        in_maps.append(m)

    if run_bass_kernel_spmd is not _REAL_RUNNER:
        # test hook (simulator path)
        res = run_bass_kernel_spmd(nc, in_maps, core_ids=list(range(cfg.n_cores)))
        results = res.results
        best = None
    else:
        results, best = _execute(nc, in_maps, cfg.n_cores, bench=bench)
    _out[0] = best

    n_out = cfg.spc * cfg.n_cores + cfg.n_nodes
    out = np.empty((n_out, cfg.ncls), np.float32)
    for c in range(cfg.n_cores):
        out[c * cfg.spc:(c + 1) * cfg.spc] = results[c]["cnn_ls"]
    nb = cfg.spc * cfg.n_cores
    core_of = pid // cfg.rows_pc
    row_of = pid % cfg.rows_pc
    labels = [results[c]["label_ls"] for c in range(cfg.n_cores)]
    lab_all = np.stack(labels)                      # (cores, rows_pc, ncls)
    out[nb:] = lab_all[core_of, row_of]
    return out

